# revision 20
# baseline (speedup 1.0000x reference)
"""Trainium2 Bass kernel for nn_BiBayesianConv.

Math (reference):
    delta = 0.5 * log(eps / (1 - eps))                    # [1,F,C,3,3]
    rw    = tanh((weight + delta) / tau)  (tau = 1.0)     # [1,F,C,3,3]
    out[s,b,f,w,h] = sum_{c,k,l} rw[s,f,c,k,l] * x[b,c,w,h]

The (k,l) sum is independent of x, so the whole module reduces to
    Weff[f,c] = sum_{k,l} tanh(weight[f,c,k,l] + delta[f,c,k,l])
    out[b,f,:] = Weff @ x[b,:,:]          # contraction over C

Weff is a tiny (512x256) elementwise+reduce — computed on host in fp32
and shipped pre-transposed as bf16 (256 KB). The device does the
68.7 GFLOP contraction.

The kernel is HBM-bound (per-core HBM ~358 GB/s vs 109 us of PE work),
so all device I/O is bf16: x is host-cast to bf16 (16 MB/core read),
the output is written bf16 (32 MB/core) and host-cast back to fp32.
~48 MB/core total vs 105 MB for the fp32 version.

Host also pre-permutes c -> (c % 128, c // 128) and f -> (f % 128,
f // 128) so every device DMA is one instruction with fully contiguous
16 KB-per-partition descriptors (max SDMA efficiency):
    x_dev[b, p, j, wh]   = x[b, j*128+p, wh]          (one 2 MB load/batch)
    out_dev[b, p, ft, wh] = out[b, ft*128+p, wh]      (one 4 MB store/batch)

Sharding: data-parallel over batch. 64 batches / 8 cores = 8 per core.

Measured on the 8 tunneled trn2 cores: ~141 us HW exec (vs 330 us f32
baseline), rel err 2.9e-3. PE matmul stream runs at its warm-clock
roofline (216 ns median spacing for N=512).
"""

import numpy as np
import ml_dtypes

import concourse.bass as bass
import concourse.mybir as mybir
import concourse.tile as tile
from concourse import bacc
from concourse.bass_utils import run_bass_kernel_spmd

# Problem shapes (hardcoded per contract).
B, C, F = 64, 256, 512
W_SP, H_SP = 64, 64
WH = W_SP * H_SP          # 4096
KL = 9                    # 3*3 kernel taps
N_CORES = 8
B_LOC = B // N_CORES      # 8 batches per core

F32 = mybir.dt.float32
BF16 = mybir.dt.bfloat16

P = 128                   # SBUF partitions
CT = C // P               # 2 c-tiles
FT = F // P               # 4 f-tiles
NCHUNK = 512              # matmul moving free dim (one PSUM bank of fp32)
NCH = WH // NCHUNK        # 8 chunks per (b, f-tile)

# Filled by kernel() after each run (BassKernelResults); test harness reads it.
LAST_RESULT = None


def _kernel_body(tc, o_d, x_d, lw_d, b_loc):
    nc = tc.nc

    with (
        tc.tile_pool(name="const", bufs=1) as cp,
        tc.tile_pool(name="xp", bufs=5) as xp,
        tc.tile_pool(name="op", bufs=3) as op,
        tc.tile_pool(name="mmps", bufs=8, space="PSUM") as pp,
    ):
        # Stationary operand: WeffT [c_part, (ct, f)] — 256 KB, on the
        # scalar ring so it overlaps batch 0's x load on the sync ring.
        lw = cp.tile([P, CT, F], BF16, tag="lw", name="lw")
        nc.scalar.dma_start(out=lw[:], in_=lw_d[:])

        # HWDGE DMAs pay ~1.5 us serialized per-instruction ring overhead:
        # keep transfers big (2-4 MB). Batch 0 is split finer so the first
        # matmul starts after 512 KB instead of 2 MB — but not finer: the
        # PE must run stall-free once started or the HAM warm-up window
        # resets and the first ~13 us run at half clock.
        def load_x(b):
            xt = xp.tile([P, CT, WH], BF16, tag="x", name="x")
            if b == 0:
                h = WH // 2
                nc.sync.dma_start(out=xt[:, 0, :h], in_=x_d[b, :, 0, :h])
                nc.sync.dma_start(out=xt[:, 0, h:], in_=x_d[b, :, 0, h:])
                nc.sync.dma_start(out=xt[:, 1], in_=x_d[b, :, 1])
            else:
                nc.sync.dma_start(out=xt[:], in_=x_d[b])
            return xt

        # out[b, f, :] = Weff @ x[b]; ct-major per (b, ft): the stationary
        # operand changes once per 8-chunk sweep and the 8 chunks land in
        # the 8 PSUM banks; evacuation of bank k overlaps the ct=1 sweep.
        def mm_block(b, ft, xt, ot, jo):
            fs = slice(ft * P, (ft + 1) * P)
            pss = []
            for ch in range(NCH):
                cs = slice(ch * NCHUNK, (ch + 1) * NCHUNK)
                ps = pp.tile([P, NCHUNK], F32, tag="mm", name=f"mm{ch}")
                nc.tensor.matmul(ps[:], lw[:, 0, fs], xt[:, 0, cs],
                                 start=True, stop=False)
                pss.append(ps)
            for ch in range(NCH):
                cs = slice(ch * NCHUNK, (ch + 1) * NCHUNK)
                nc.tensor.matmul(pss[ch][:], lw[:, 1, fs], xt[:, 1, cs],
                                 start=False, stop=True)
                # balance PSUM evacuation (fp32 -> bf16 cast) across DVE/ACT
                if ch % 2 == 0:
                    nc.vector.tensor_copy(out=ot[:, jo, cs], in_=pss[ch][:])
                else:
                    nc.scalar.copy(out=ot[:, jo, cs], in_=pss[ch][:])

        # loads own the sync ring; stores alternate between the scalar
        # HWDGE ring and the gpsimd SWDGE ring (independent issue paths —
        # halves per-ring serialization overhead). Big 4 MB stores in the
        # body amortize the ~2 us per-DMA ring overhead; the last two
        # batches drain progressively finer so the post-matmul tail is
        # just one 512 KB store deep on each ring.
        for b in range(b_loc):
            xt = load_x(b)
            ot = op.tile([P, FT, WH], BF16, tag="ot", name="ot")
            for ft in range(FT):
                mm_block(b, ft, xt, ot, ft)
                if b == b_loc - 2:
                    if ft == 1:
                        nc.scalar.dma_start(out=o_d[b, :, 0:2], in_=ot[:, 0:2])
                    elif ft == 3:
                        nc.gpsimd.dma_start(out=o_d[b, :, 2:4], in_=ot[:, 2:4])
                elif b == b_loc - 1:
                    if ft < 3:
                        seng = nc.scalar if ft % 2 == 0 else nc.gpsimd
                        seng.dma_start(out=o_d[b, :, ft], in_=ot[:, ft])
                    else:
                        h = WH // 2
                        nc.gpsimd.dma_start(out=o_d[b, :, 3, :h],
                                            in_=ot[:, 3, :h])
                        nc.scalar.dma_start(out=o_d[b, :, 3, h:],
                                            in_=ot[:, 3, h:])
            if b < b_loc - 2:
                seng = nc.scalar if b % 2 == 0 else nc.gpsimd
                seng.dma_start(out=o_d[b], in_=ot[:])


def build_nc(b_loc=B_LOC):
    nc = bacc.Bacc(trn_type="TRN2", target_bir_lowering=False, debug=False)
    x_d = nc.dram_tensor("x", [b_loc, P, CT, WH], BF16,
                         kind="ExternalInput").ap()
    lw_d = nc.dram_tensor("lweff", [P, CT, F], BF16, kind="ExternalInput").ap()
    o_d = nc.dram_tensor("out", [b_loc, P, FT, WH], BF16,
                         kind="ExternalOutput").ap()
    with tile.TileContext(nc) as tc:
        _kernel_body(tc, o_d, x_d, lw_d, b_loc)
    nc.compile()
    return nc


def kernel(x, weight, epsilon):
    """Full inputs in, full output out. Shards batch across 8 NeuronCores."""
    global LAST_RESULT
    x = np.ascontiguousarray(x, dtype=np.float32).reshape(B, C, WH)
    w = np.asarray(weight, dtype=np.float32).reshape(F, C, KL)
    e = np.asarray(epsilon, dtype=np.float32).reshape(F, C, KL)

    # Weff[f,c] = sum_kl tanh(w + 0.5*(ln e - ln(1-e))); tiny, host-side.
    delta = 0.5 * (np.log(e) - np.log1p(-e))
    weff = np.tanh(w + delta).sum(axis=2)            # [F, C]
    # [c, f] -> [c % 128, c // 128, f], bf16
    lweff = np.ascontiguousarray(
        weff.T.reshape(CT, P, F).transpose(1, 0, 2)
    ).astype(ml_dtypes.bfloat16)
    # [b, c, wh] -> [b, c % 128, c // 128, wh], bf16
    x_bf = np.ascontiguousarray(
        x.reshape(B, CT, P, WH).transpose(0, 2, 1, 3)
    ).astype(ml_dtypes.bfloat16)

    nc = build_nc()
    in_maps = [
        {"x": x_bf[i * B_LOC:(i + 1) * B_LOC], "lweff": lweff}
        for i in range(N_CORES)
    ]
    res = run_bass_kernel_spmd(nc, in_maps, core_ids=list(range(N_CORES)))
    LAST_RESULT = res
    # device layout [b, f % 128, f // 128, wh] -> [b, f, w, h], fp32
    out = np.concatenate([r["out"] for r in res.results], axis=0)
    out = out.transpose(0, 2, 1, 3).reshape(B, F, W_SP, H_SP).astype(np.float32)
    return out[None]  # [1, B, F, W, H]
